# revision 22
# baseline (speedup 1.0000x reference)
"""Trainium2 Bass kernel for nn_BasicLSTM: (B,T,N,C) shared-weight LSTM -> FC.

Strategy (data parallel over 8 cores, B=64 -> 8 batches/core):
  - seqs = 8*1370 = 10960 independent (b,n) sequences per core, T=12, C=8, H=64.
  - Pairs of two 512-seq blocks share one rhs tile [73, 1024]; per step, 8
    matmuls (4 gates x 2 blocks) with stationary lhsT = [W_hh.T; W_ih.T; b]
    (73 x 64) write PSUM [128, 2048] (block0 -> partitions 0:64, block1 ->
    64:128; gate G in cols G*512:(G+1)*512, order i,f,o,g).
  - The g-gate weights/bias are doubled on the host so ONE Sigmoid per pair
    over the whole [128, 2048] PSUM tile yields i,f,o and g' = sigmoid(2x)
    = (tanh(x)+1)/2.  This is the kernel's wall: ~42M sigmoid/tanh elements
    must pass through the one Activation engine (~153G elem/s => ~315us).
  - Sigmoid output is written STRIDED into a gate-major sbuf tile
    [128, 4, 1024] shared by a superpair (2 pairs), so every DVE op below is
    a contiguous [128, 1024] slice (contiguous => DVE 2x mode; strided
    operands drop to 1x).
  - c' = f*c + i*(2g'-1): the (2g'-1) fix-up runs on the otherwise-idle
    GpSimd engine (tensor_scalar); i*gf, f*c, add on DVE.  tanh(c) is one
    Activation op per QUAD (2 superpairs share a [128, 2048] c tile),
    amortizing the ~270ns ACT fixed cost.
  - h = sigmoid(o) * tanh(c) is written DIRECTLY into the next step's rhs
    tiles: block0 via a plain [64,512] DVE multiply, block1 via a
    partition-SHIFTED DVE multiply (reads partitions 64:128, writes 0:64)
    -- no scratch tile, no SBUF-SBUF DMA, no sync-queue semaphores.
  - All 11 pairs' recurrences are interleaved in one rotation so the
    ~6us per-step dependency chain hides under ~28us of engine work per
    round.  FC for a finished superpair is emitted two superpairs later so
    its PSUM-slot reuse never blocks fresh gate matmuls.
  - x arrives pre-transposed from the host as (9, T, seqs) bf16 (channel 8 is
    the constant 1.0 row that carries the biases through the contraction).
  - FC tail: y = W_fc @ h_T + b_fc as K=65 matmuls over the stored final h.
"""

import os
from contextlib import ExitStack

import numpy as np

import concourse.bass as bass
import concourse.mybir as mybir
import concourse.tile as tile
from concourse import bacc
from concourse.bass_utils import run_bass_kernel_spmd

B, T, N, C, H = 64, 12, 1370, 8, 64
NCORES = 8
BPC = B // NCORES          # batches per core
SEQS = BPC * N             # 10960 sequences per core
S = 512                    # block size (free dim per matmul)
PW = 2 * S                 # pair width (two blocks per rhs tile)
KH = H                     # 64 rows of h in rhs
KX = C + 1                 # 8 x-channels + ones row
K = KH + KX                # 73
G4 = 4 * H                 # 256

BF16 = mybir.dt.bfloat16
F32 = mybir.dt.float32
CDT = BF16  # cell-state dtype (bf16 doubles DVE throughput)
NPBF16 = mybir.dt.np(BF16)

AF = mybir.ActivationFunctionType
ALU = mybir.AluOpType

# gate order in wg / PSUM columns: i, f, o, g  (g last, with 2x baked in)
GI, GF, GO, GG = 0, 1, 2, 3


def _pairs(seqs: int):
    """[(c0, w0, w1)] with w0 the width of block0 (<=S), w1 of block1."""
    out = []
    c0 = 0
    while c0 < seqs:
        w = min(PW, seqs - c0)
        w0 = min(S, w)
        w1 = w - w0
        out.append((c0, w0, w1))
        c0 += w
    return out


def build_nc(seqs: int = SEQS) -> bass.Bass:
    nc = bacc.Bacc("TRN2", target_bir_lowering=False, debug=False)

    xin = nc.declare_dram_parameter("xin", [KX, T, seqs], BF16, isOutput=False)
    wg = nc.declare_dram_parameter("wg", [K, G4], BF16, isOutput=False)
    wfc = nc.declare_dram_parameter("wfc", [H + 1, C], BF16, isOutput=False)
    y = nc.declare_dram_parameter("y", [C, seqs], F32, isOutput=True)

    pairs = _pairs(seqs)                       # 11 pairs for 10960
    sps = [pairs[i : i + 2] for i in range(0, len(pairs), 2)]  # superpairs

    with tile.TileContext(nc) as tc, ExitStack() as ctx:
        const = ctx.enter_context(tc.tile_pool(name="const", bufs=1))
        rhsp = ctx.enter_context(tc.tile_pool(name="rhs", bufs=36))
        sgp = ctx.enter_context(tc.tile_pool(name="sg", bufs=6))
        upool = ctx.enter_context(tc.tile_pool(name="u", bufs=4))
        vpool = ctx.enter_context(tc.tile_pool(name="v", bufs=4))
        wpool = ctx.enter_context(tc.tile_pool(name="w", bufs=4))
        cstp = ctx.enter_context(tc.tile_pool(name="cst", bufs=3))
        thcp = ctx.enter_context(tc.tile_pool(name="thc", bufs=3))
        ysp = ctx.enter_context(tc.tile_pool(name="ys", bufs=4))
        pgp = ctx.enter_context(tc.tile_pool(name="pg", bufs=2, space="PSUM"))

        w_sb = const.tile([K, G4], BF16)
        nc.sync.dma_start(out=w_sb[:, :], in_=wg[:, :])
        wfc_sb = const.tile([H + 1, C], BF16)
        hstore = const.tile([H + 1, seqs], BF16)
        # PE warm-up + early ACT table load, all under the initial DMA shadow
        scratch = const.tile([128, S], BF16)
        nc.vector.memset(scratch[0:64, 0:64], 1.0)
        nc.scalar.activation(scratch[0:1, 0:8], scratch[0:1, 0:8], AF.Sigmoid)
        # spin the PE clock out of its cold p-state before the real matmuls
        pwarm = pgp.tile([128, 4 * S], F32, tag="pg", name="pwarm")
        for i in range(32):
            nc.tensor.matmul(pwarm[0:64, 0:64], scratch[0:64, 0:64], scratch[0:64, 0:64])

        def alloc_rhs(st, t):
            """One rhs tile per pair; x rows for both blocks in one DMA."""
            for p, (c0, w0, w1) in enumerate(st["pairs"]):
                r = rhsp.tile([K, PW], BF16, name="rhs", tag="rhs")
                st["rhs"][t][p] = r
                nc.sync.dma_start(
                    out=r[KH:K, 0 : w0 + w1], in_=xin[:, t, c0 : c0 + w0 + w1]
                )

        def emit_gates(st, t):
            prs, rhs_t = st["pairs"], st["rhs"][t]
            cst, coff = st["c"]
            np_ = len(prs)
            # gate-major sg: [128, gate, pair*S] so every DVE operand below is
            # a contiguous [128, 1024] slice
            sg = sgp.tile([128, 4, 2 * S], BF16, name="sg", tag="sg")
            st["sg"] = sg

            # matmuls, gate-major across the superpair (lhsT reuse), then one
            # sigmoid per pair over its whole [128, 2048] PSUM tile, written
            # strided into the gate-major sg layout
            pg = [
                pgp.tile([128, 4 * S], F32, name="pg", tag="pg") for _ in range(np_)
            ]
            for g in (GI, GF, GO, GG):
                for p, (c0, w0, w1) in enumerate(prs):
                    for blk, (cb, wd) in enumerate(((0, w0), (S, w1))):
                        if wd == 0:
                            continue
                        pb = 64 * blk
                        if t == 0:
                            lh = w_sb[KH:K, g * H : g * H + H]
                            rh = rhs_t[p][KH:K, cb : cb + wd]
                        else:
                            lh = w_sb[:, g * H : g * H + H]
                            rh = rhs_t[p][:, cb : cb + wd]
                        nc.tensor.matmul(
                            pg[p][pb : pb + 64, g * S : g * S + wd], lh, rh
                        )
            for p in range(np_):
                nc.scalar.activation(
                    sg[:, :, p * S : p * S + S], pg[p][:, :], AF.Sigmoid
                )

            W2 = np_ * S
            cs = cst[:, coff : coff + W2]
            # c' = f*c + i*(2g'-1); gfix on GpSimd, rest contiguous on DVE
            gf = vpool.tile([128, 2 * S], BF16, name="gf", tag="gf")
            nc.gpsimd.tensor_scalar(
                gf[:, 0:W2], sg[:, GG, 0:W2], 2.0, -1.0, op0=ALU.mult, op1=ALU.add
            )
            u = upool.tile([128, 2 * S], BF16, name="u", tag="u")
            if t == 0:
                nc.vector.tensor_mul(cs, sg[:, GI, 0:W2], gf[:, 0:W2])
            else:
                nc.vector.tensor_mul(u[:, 0:W2], sg[:, GI, 0:W2], gf[:, 0:W2])
                w_ = wpool.tile([128, 2 * S], CDT, name="w", tag="w")
                nc.vector.tensor_mul(w_[:, 0:W2], sg[:, GF, 0:W2], cs)
                nc.vector.tensor_add(cs, u[:, 0:W2], w_[:, 0:W2])

        def emit_h(st, t, thc, coff):
            # h = sigmoid(o) * tanh(c); block1's h written straight into rhs
            # cols 512:1024 via a partition-shifted DVE multiply (no DMA)
            sg = st["sg"]
            for p, (c0, w0, w1) in enumerate(st["pairs"]):
                o_lo = sg[0:64, GO, p * S : p * S + w0]
                t_lo = thc[0:64, coff + p * S : coff + p * S + w0]
                if t == T - 1:
                    d_lo = hstore[0:H, c0 : c0 + w0]
                else:
                    d_lo = st["rhs"][t + 1][p][0:KH, 0:w0]
                nc.vector.tensor_mul(d_lo, o_lo, t_lo)
                if w1 > 0:
                    o_hi = sg[64:128, GO, p * S : p * S + w1]
                    t_hi = thc[64:128, coff + p * S : coff + p * S + w1]
                    if t == T - 1:
                        d_hi = hstore[0:H, c0 + w0 : c0 + w0 + w1]
                    else:
                        d_hi = st["rhs"][t + 1][p][0:KH, S : S + w1]
                    nc.vector.tensor_mul(d_hi, o_hi, t_hi)

        # interleave ALL superpairs so every engine always has independent
        # recurrence work to hide the per-step dependency chain
        PREFETCH = 2

        states = []
        quads = []
        for iq in range(0, len(sps), 2):
            qsps = sps[iq : iq + 2]
            qw = sum(len(sp) for sp in qsps) * S
            cst = cstp.tile([128, 4 * S], CDT, name="c_t", tag="c_t")
            qstates = []
            coff = 0
            for sp in qsps:
                st = {"pairs": sp, "rhs": [[None] * len(sp) for _ in range(T)],
                      "c": (cst, coff)}
                coff += len(sp) * S
                states.append(st)
                qstates.append(st)
                for t in range(PREFETCH):
                    alloc_rhs(st, t)
            quads.append({"sts": qstates, "cst": cst, "qw": qw})

        # deferred const loads: only needed by the FC tail, so they queue
        # behind the first x prefetches instead of ahead of them
        nc.sync.dma_start(out=wfc_sb[:, :], in_=wfc[:, :])
        # ones row comes from xin's ones channel (avoids a slow gpsimd memset)
        nc.sync.dma_start(out=hstore[H : H + 1, :], in_=xin[C : C + 1, 0, :])

        def emit_fc(st):
            for c0, w0, w1 in st["pairs"]:
                for cb, wd in ((c0, w0), (c0 + w0, w1)):
                    if wd == 0:
                        continue
                    pf = pgp.tile([128, 4 * S], F32, tag="pg", name="pf")
                    nc.tensor.matmul(
                        pf[0:C, 0:wd], wfc_sb[:, :], hstore[:, cb : cb + wd]
                    )
                    yt = ysp.tile([C, S], F32, name="yt", tag="yt")
                    nc.vector.tensor_copy(yt[:, 0:wd], pf[0:C, 0:wd])
                    nc.sync.dma_start(out=y[:, cb : cb + wd], in_=yt[:, 0:wd])

        # FC for superpair k is emitted two superpairs later so its PSUM-slot
        # reuse never chains fresh gate matmuls behind a finished h-chain
        fc_queue = []
        for t in range(T):
            for q in quads:
                if t == T - 1:
                    # drain round: per-superpair tanh decouples the final
                    # chains so the last FC isn't serialized across the quad
                    for st in q["sts"]:
                        emit_gates(st, t)
                        cst, coff = st["c"]
                        w2 = len(st["pairs"]) * S
                        thc = thcp.tile([128, 4 * S], BF16, name="thc", tag="thc")
                        nc.scalar.activation(
                            thc[:, 0:w2], cst[:, coff : coff + w2], AF.Tanh
                        )
                        emit_h(st, t, thc, 0)
                        fc_queue.append(st)
                        if len(fc_queue) > 2:
                            emit_fc(fc_queue.pop(0))
                    continue
                for st in q["sts"]:
                    if t + PREFETCH < T:
                        alloc_rhs(st, t + PREFETCH)
                    emit_gates(st, t)
                thc = thcp.tile([128, 4 * S], BF16, name="thc", tag="thc")
                nc.scalar.activation(
                    thc[:, 0 : q["qw"]], q["cst"][:, 0 : q["qw"]], AF.Tanh
                )
                coff = 0
                for st in q["sts"]:
                    emit_h(st, t, thc, coff)
                    coff += len(st["pairs"]) * S
        for st in fc_queue:
            emit_fc(st)

    nc.compile()
    return nc


def prep_inputs(x, W_ih, W_hh, b_ih, b_hh, W_fc, b_fc, seqs=SEQS, ncores=NCORES):
    """Host-side shard + transpose + weight packing. Returns in_maps."""
    x = np.asarray(x, dtype=np.float32)
    W_ih = np.asarray(W_ih, dtype=np.float32)
    W_hh = np.asarray(W_hh, dtype=np.float32)
    b = np.asarray(b_ih, dtype=np.float32) + np.asarray(b_hh, dtype=np.float32)
    W_fc = np.asarray(W_fc, dtype=np.float32)
    b_fc = np.asarray(b_fc, dtype=np.float32)

    # pytorch param gate order: i, f, g, o -> our column order i, f, o, g
    # (g last, scaled by 2 so sigmoid(2x) = (tanh(x)+1)/2)
    SRC = {GI: 0, GF: 1, GO: 3, GG: 2}
    wg = np.zeros((K, G4), dtype=np.float32)
    for g in range(4):
        rows = slice(H * SRC[g], H * SRC[g] + H)
        scale = 2.0 if g == GG else 1.0
        wg[0:KH, H * g : H * g + H] = scale * W_hh[rows, :].T
        wg[KH : KH + C, H * g : H * g + H] = scale * W_ih[rows, :].T
        wg[K - 1, H * g : H * g + H] = scale * b[rows]
    wg = wg.astype(NPBF16)

    wfc = np.concatenate([W_fc.T, b_fc[None, :]], axis=0).astype(NPBF16)  # (65, 8)

    bpc = x.shape[0] // ncores
    in_maps = []
    for k in range(ncores):
        xc = x[k * bpc : (k + 1) * bpc]              # (bpc, T, N, C)
        xt = xc.transpose(3, 1, 0, 2).reshape(C, T, seqs)
        xext = np.empty((KX, T, seqs), dtype=NPBF16)
        xext[0:C] = xt.astype(NPBF16)
        xext[C] = np.ones((T, seqs), dtype=NPBF16)
        in_maps.append({"xin": xext, "wg": wg, "wfc": wfc})
    return in_maps


_CACHE = {}


def _get_nc():
    if "nc" not in _CACHE:
        _CACHE["nc"] = build_nc()
    return _CACHE["nc"]


def kernel(x, W_ih, W_hh, b_ih, b_hh, W_fc, b_fc, **run_kwargs):
    nc = _get_nc()
    in_maps = prep_inputs(x, W_ih, W_hh, b_ih, b_hh, W_fc, b_fc)
    res = run_bass_kernel_spmd(nc, in_maps, list(range(NCORES)), **run_kwargs)
    outs = res.results
    ys = []
    for k in range(NCORES):
        yk = np.asarray(outs[k]["y"])               # (C, SEQS) f32
        ys.append(yk.T.reshape(BPC, N, C))
    y = np.concatenate(ys, axis=0)                  # (B, N, C)
    if run_kwargs.get("trace"):
        _CACHE["last_result"] = res
    return y.astype(np.float32)
